# revision 8
# baseline (speedup 1.0000x reference)
"""Trainium2 Bass kernel for CompactCenterLoss (B=4096, D=512, C=100, 8 cores).

Math notes (vs the reference):
  dist[i, j] = ||x_i - centers[t_j]|| depends on j only through the class
  t_j, so the first BxB table collapses to a [B, C] table D2:
      dist_ap[i] = D2[i, t_i]                      (all same-class j equal)
      dist_an[i] = min_{c present, c != t_i} D2[i, c]
  Only pdist(x, x) needs the full BxB compute. Its masked row sums are
  obtained per class via matmuls with the one-hot matrix O [B, C]:
      S[i, c] = sum_j O[j, c] * dist[j, i]
      pos_sum_i = S[i, t_i],  tot_sum_i = sum_c S[i, c]

Sharding: batch rows are split across 8 cores (512 rows each). Every core
computes dist^T tiles [128 j x 512 i_shard] against the full (replicated)
input, using a per-core ROTATION of the j axis so the diagonal block is
always device-j-tiles 0..3 -- a single uniform SPMD program for all cores.

Device schedule (per core), tuned against the TimelineSim cost model
(matmul cost = out_free_cols x cycles_per_row; fp8 DoubleRow = 0.5
cyc/row, bf16 = 1, fp32 = 4; ACT = 1 elem/cycle @ 1.2 GHz):
  - Gram in fp8 DoubleRow: 2 DR matmuls per [128, 512] tile (K=512).
  - BOTH squared-norm terms fold into the same PSUM group via one extra
    fp8 DR rank-4 matmul per tile (hi/lo fp8 split of -0.5|x_j|^2 paired
    with ones, and ones paired with hi/lo of -0.5|x_i|^2), so the sqrt is
    bias-free and runs PAIRED on ACT ([128, 2, 512] per instruction).
  - sqrt writes fp8 dist tiles straight into one persistent [128, 32, 512]
    fp8 buffer; the per-class row sums S accumulate as 4 interleaved PSUM
    groups [128 i, 100] in a single PSUM bank via flipped fp8 DR matmuls
    (dist pair = stationary, one-hot pair = moving, 100-col output), one
    pair of j-tiles per matmul quartet, lagged 3 pairs behind the Gram.
  - The diagonal sqrt argument is pushed to DIAG_SQ = 4096 (DVE adds
    -2048*I to the PSUM block, ACT re-sqrts the [128,128] sub-block), so
    the diagonal of dist is EXACTLY 64.0 in fp8; the host subtracts 64
    from pos_sum/tot_sum. No on-device diagonal masking needed.
  - Phase A (the [B, C] center table deciding loss1/prec) runs in bf16
    hi/lo double-double (3 cross terms, error ~1e-5 << the 2.3e-3 min
    an/ap margin), with the norm terms as a bf16 hi/lo K=4 rank update.
  - All inputs are host-prepacked into SBUF layout (one DMA each); the
    finished S ships to the host straight from PSUM.

Accuracy vs the f32 reference on the benchmark data: loss rel err ~1e-3
(fp8 quantization of the distance table averages out over the per-class
sums), prec exact (decided by the bf16-hi/lo phase A, err ~1e-5).
"""

import numpy as np
import ml_dtypes
from contextlib import ExitStack

import jax
import concourse.bass as bass
import concourse.tile as tile
import concourse.mybir as mybir
from concourse import bacc
from concourse.bass2jax import install_neuronx_cc_hook, _bass_exec_p, partition_id_tensor

B, D, C = 4096, 512, 100
N_CORES = 8
P = 128
SH = B // N_CORES          # 512 rows per core
NJ = B // P                # 32 j-tiles
KT = D // P                # 4 k-tiles
NCH = SH // P              # 4 i-chunks per core
NP_ = NJ // 2              # 16 j-tile pairs
# xt column chunks (narrow first chunk -> compute starts sooner)
CC_BOUNDS = [0, 512, 1024, 2048, 3072, 4096]
NCC = len(CC_BOUNDS) - 1
BIG = 1.0e12
DIAG_SQ = 4096.0
SQ_SCALE = 1.0625   # dist pre-scale: centers the device's truncating fp8
                    # quantization (err in [-ulp,0] -> ~±ulp/2); host divides it out  # sqrt arg forced onto the diagonal: sqrt = 64.0 exactly in
                  # fp8 (|arg noise| < 512), host subtracts 64 from the sums
IAML_MARGIN = 5.0
CP = 128            # one-hot padded to 128 classes (dual-fp8 ldweights wants m=128)

f32 = mybir.dt.float32
bf16 = mybir.dt.bfloat16
fp8 = mybir.dt.float8e4
BF16_NP = ml_dtypes.bfloat16
FP8_NP = ml_dtypes.float8_e4m3

Alu = mybir.AluOpType
Act = mybir.ActivationFunctionType


def _build_program():
    nc = bacc.Bacc("TRN2", target_bir_lowering=False, debug=False,
                   enable_asserts=True, num_devices=1)

    # ---- DRAM I/O (per core; host pre-rotates the j axis by the shard offset
    # and prepacks every tensor into its SBUF layout)
    # X^T in fp8, cols rotated, laid out for DoubleRow: [g, p, s, n] holds
    # element k = g*256 + s*128 + p of column n
    xt8_d = nc.dram_tensor("xt8", [2, P, 2, B], fp8, kind="ExternalInput")
    # norm-fold DR operands: xe8 [p, s, B] = [[hi_j, 1], [lo_j, 1]],
    # xr8 [p, s, SH] = [[1, hi_i], [1, lo_i]]  (hi/lo fp8 of -0.5|x|^2)
    xe8_d = nc.dram_tensor("xe8", [2, 2, B], fp8, kind="ExternalInput")
    xr8_d = nc.dram_tensor("xr8", [2, 2, SH], fp8, kind="ExternalInput")
    ohp_d = nc.dram_tensor("ohp", [P, NJ * CP], fp8, kind="ExternalInput")  # one-hot, j-tiled+rotated, class-padded
    nI_d = nc.dram_tensor("nI", [P, P], bf16, kind="ExternalInput")         # -0.5*DIAG_SQ*I
    # phase A: bf16 hi/lo of X_shard^T and centers^T, prepacked to SBUF layout
    xhi_d = nc.dram_tensor("xhi", [P, KT * SH], bf16, kind="ExternalInput")
    xlo_d = nc.dram_tensor("xlo", [P, KT * SH], bf16, kind="ExternalInput")
    cthl_d = nc.dram_tensor("cthl", [P, KT * 2 * C], bf16, kind="ExternalInput")
    # phase A norm rows: cols 0:SH = [hiA, loA, 1, 1] (A = -0.5|x_i|^2),
    # cols SH: = [1, 1, hiB, loB] (B = -0.5(|c|^2 + BIG*absent))
    xab_d = nc.dram_tensor("xab", [4, SH + C], bf16, kind="ExternalInput")
    ohs_d = nc.dram_tensor("ohs", [P, NCH * C], f32, kind="ExternalInput")   # one-hot shard, chunk-tiled
    ohsb_d = nc.dram_tensor("ohsb", [P, NCH * C], f32, kind="ExternalInput")  # BIG * same
    out_d = nc.dram_tensor("out", [P, 2 * NCH], f32, kind="ExternalOutput")
    out2_d = nc.dram_tensor("out2", [C, SH], f32, kind="ExternalOutput")       # S^T

    with tile.TileContext(nc) as tc, ExitStack() as ctx:
        singles = ctx.enter_context(tc.tile_pool(name="singles", bufs=1))
        tmp = ctx.enter_context(tc.tile_pool(name="tmp", bufs=3))
        gram_pool = ctx.enter_context(tc.tile_pool(name="gram", bufs=3, space="PSUM"))
        g1_pool = ctx.enter_context(tc.tile_pool(name="g1", bufs=1, space="PSUM"))
        s_pool = ctx.enter_context(tc.tile_pool(name="sacc", bufs=1, space="PSUM"))

        # ---- DMAs in consumption order
        xt_sb = [[None] * NCC for _ in range(2)]

        def load_xt(cc):
            lo, hi = CC_BOUNDS[cc], CC_BOUNDS[cc + 1]
            for g in range(2):
                t_ = singles.tile([P, 2, hi - lo], fp8, tag=f"xt{g}_{cc}",
                                  name=f"xt{g}_{cc}")
                nc.sync.dma_start(out=t_, in_=xt8_d.ap()[g, :, :, lo:hi])
                xt_sb[g][cc] = t_

        load_xt(0)
        xe8_sb = singles.tile([2, 2, B], fp8, tag="xe8")
        nc.sync.dma_start(out=xe8_sb, in_=xe8_d.ap())
        xr8_sb = singles.tile([2, 2, SH], fp8, tag="xr8")
        nc.sync.dma_start(out=xr8_sb, in_=xr8_d.ap())
        nI_sb = singles.tile([P, P], bf16, tag="nI")
        nc.sync.dma_start(out=nI_sb, in_=nI_d.ap())
        ohp_sb = singles.tile([P, NJ, CP], fp8, tag="ohp")
        nc.sync.dma_start(out=ohp_sb,
                          in_=ohp_d.ap().rearrange("p (t c) -> p t c", t=NJ))

        load_xt(1)
        load_xt(2)
        load_xt(3)
        # phase A inputs (consumed mid-stream, before the last xt chunk)
        xhi_sb = singles.tile([P, KT, SH], bf16, tag="xhi")
        nc.sync.dma_start(out=xhi_sb,
                          in_=xhi_d.ap().rearrange("p (k s) -> p k s", k=KT))
        xlo_sb = singles.tile([P, KT, SH], bf16, tag="xlo")
        nc.sync.dma_start(out=xlo_sb,
                          in_=xlo_d.ap().rearrange("p (k s) -> p k s", k=KT))
        cthl_sb = singles.tile([P, KT, 2, C], bf16, tag="cthl")
        nc.sync.dma_start(out=cthl_sb,
                          in_=cthl_d.ap().rearrange("p (k t c) -> p k t c",
                                                    k=KT, t=2))
        xab_sb = singles.tile([4, SH + C], bf16, tag="xab")
        nc.sync.dma_start(out=xab_sb, in_=xab_d.ap())
        ohs_sb = singles.tile([P, NCH, C], f32, tag="ohs")
        nc.sync.dma_start(out=ohs_sb, in_=ohs_d.ap().rearrange("p (k c) -> p k c", k=NCH))
        ohsb_sb = singles.tile([P, NCH, C], f32, tag="ohsb")
        nc.sync.dma_start(out=ohsb_sb, in_=ohsb_d.ap().rearrange("p (k c) -> p k c", k=NCH))

        load_xt(4)

        out_sb = singles.tile([P, 2 * NCH], f32, tag="out")
        dist8 = singles.tile([P, NJ, SH], fp8, tag="dist8")

        # ---- phase B: pdist(x, x) tiles + per-class row sums S
        st_acc = s_pool.tile([CP, SH], f32, tag="st", name="st_acc")

        def s_matmul(u):
            nc.tensor.matmul(st_acc,
                             ohp_sb[:, 2 * u:2 * u + 2, :],
                             dist8[:, 2 * u:2 * u + 2, :],
                             start=(u == 0), stop=(u == NP_ - 1),
                             perf_mode=mybir.MatmulPerfMode.DoubleRow,
                             skip_group_check=True)

        def emit_phase_a():
            # phase A emitted mid-stream: its matmuls interleave with phase
            # B's (the PE is warm) and its DVE/ACT epilogue hides under the
            # remaining pairs instead of serializing at the kernel tail.
            d2a = singles.tile([P, NCH, C], f32, tag="d2a")
            g1 = g1_pool.tile([P, NCH, C], f32, name="g1")  # one PSUM bank
            for k in range(NCH):
                sl = slice(k * P, (k + 1) * P)
                for kt in range(KT):
                    nc.tensor.matmul(g1[:, k, :], xhi_sb[:, kt, sl],
                                     cthl_sb[:, kt, 0, :],
                                     start=(kt == 0), stop=False,
                                     skip_group_check=True)
                for kt in range(KT):
                    nc.tensor.matmul(g1[:, k, :], xhi_sb[:, kt, sl],
                                     cthl_sb[:, kt, 1, :],
                                     start=False, stop=False,
                                     skip_group_check=True)
                for kt in range(KT):
                    nc.tensor.matmul(g1[:, k, :], xlo_sb[:, kt, sl],
                                     cthl_sb[:, kt, 0, :],
                                     start=False, stop=False,
                                     skip_group_check=True)
                nc.tensor.matmul(g1[:, k, :], xab_sb[:, sl],
                                 xab_sb[:, SH:SH + C], start=False, stop=True,
                                 skip_group_check=True)
            nc.scalar.activation(out=d2a, in_=g1, func=Act.Sqrt,
                                 bias=0.0, scale=-2.0)
            jk = tmp.tile([P, NCH, C], f32, tag="jk")
            ap = tmp.tile([P, NCH], f32, tag="ap")
            nc.vector.tensor_mul(jk, d2a, ohs_sb)
            nc.vector.tensor_reduce(ap, jk, axis=mybir.AxisListType.X,
                                    op=Alu.add)
            jk2 = tmp.tile([P, NCH, C], f32, tag="jk2")
            an = tmp.tile([P, NCH], f32, tag="an")
            nc.vector.tensor_add(jk2, d2a, ohsb_sb)
            nc.vector.tensor_reduce(an, jk2, axis=mybir.AxisListType.X,
                                    op=Alu.min)
            diff = tmp.tile([P, NCH], f32, tag="diff")
            nc.vector.tensor_sub(diff, ap, an)
            nc.vector.tensor_scalar_max(out_sb[:, 0:NCH], diff, 0.0)
            nc.vector.tensor_tensor(out=out_sb[:, NCH:2 * NCH],
                                    in0=an, in1=ap, op=Alu.is_gt)
            nc.sync.dma_start(out=out_d.ap(), in_=out_sb)

        for pr in range(NP_):
            gram2 = gram_pool.tile([P, 2, SH], f32)
            for s in range(2):
                t = 2 * pr + s
                col = t * P
                cc = next(i for i in range(NCC)
                          if CC_BOUNDS[i] <= col < CC_BOUNDS[i + 1])
                col0 = col - CC_BOUNDS[cc]
                for g in range(2):
                    nc.tensor.matmul(gram2[:, s, :],
                                     xt_sb[g][cc][:, :, col0:col0 + P],
                                     xt_sb[g][0][:, :, 0:SH],
                                     start=(g == 0), stop=False,
                                     perf_mode=mybir.MatmulPerfMode.DoubleRow)
                # both norms as one fp8 DR rank-4 update
                nc.tensor.matmul(gram2[:, s, :],
                                 xe8_sb[:, :, t * P:(t + 1) * P],
                                 xr8_sb, start=False, stop=True,
                                 perf_mode=mybir.MatmulPerfMode.DoubleRow)
            if pr < 2:
                # diagonal blocks (tiles 0..3): push the sqrt argument to
                # +DIAG_SQ in-place in PSUM (DVE) so the paired sqrt below
                # lands the diagonal on exactly 64.0 in fp8 -- no re-sqrt,
                # and the sqrt argument is never negative
                for s in range(2):
                    t = 2 * pr + s
                    nc.vector.tensor_add(gram2[:, s, t * P:(t + 1) * P],
                                         gram2[:, s, t * P:(t + 1) * P],
                                         nI_sb)
            if pr >= 3:
                s_matmul(pr - 3)
            nc.scalar.activation(out=dist8[:, 2 * pr:2 * pr + 2, :],
                                 in_=gram2, func=Act.Sqrt,
                                 bias=0.0, scale=-2.0 * SQ_SCALE * SQ_SCALE)
            if pr == NP_ - 5:
                emit_phase_a()
        s_matmul(NP_ - 3)
        s_matmul(NP_ - 2)
        s_matmul(NP_ - 1)

        # ---- tail: ship S^T; host finishes loss2
        sts = singles.tile([C, SH], f32, tag="sts")
        nc.vector.tensor_copy(sts, st_acc[0:C, :])
        nc.sync.dma_start(out=out2_d.ap(), in_=sts)

    nc.compile()
    return nc


_RUNNER = None


def _make_runner():
    """Build the program once and return a cached callable
    in_maps -> list of per-core {"out": ..., "out2": ...}. Mirrors
    concourse.bass2jax.run_bass_via_pjrt but keeps the jitted executable
    alive so repeated kernel() calls don't recompile."""
    from jax.sharding import Mesh, PartitionSpec
    from jax.experimental.shard_map import shard_map

    nc = _build_program()
    install_neuronx_cc_hook()

    partition_name = nc.partition_id_tensor.name if nc.partition_id_tensor else None
    in_names, out_names, out_avals, zero_shapes = [], [], [], []
    for alloc in nc.m.functions[0].allocations:
        if not isinstance(alloc, mybir.MemoryLocationSet):
            continue
        name = alloc.memorylocations[0].name
        if alloc.kind == "ExternalInput":
            if name != partition_name:
                in_names.append(name)
        elif alloc.kind == "ExternalOutput":
            shape = tuple(alloc.tensor_shape)
            dtype = mybir.dt.np(alloc.dtype)
            out_names.append(name)
            out_avals.append(jax.core.ShapedArray(shape, dtype))
            zero_shapes.append((shape, dtype))
    n_params = len(in_names)
    n_outs = len(out_avals)
    all_in_names = list(in_names) + list(out_names)
    if partition_name is not None:
        all_in_names.append(partition_name)
    donate = tuple(range(n_params, n_params + n_outs))

    def _body(*args):
        operands = list(args)
        if partition_name is not None:
            operands.append(partition_id_tensor())
        outs = _bass_exec_p.bind(
            *operands,
            out_avals=tuple(out_avals),
            in_names=tuple(all_in_names),
            out_names=tuple(out_names),
            lowering_input_output_aliases=(),
            sim_require_finite=True,
            sim_require_nnan=True,
            nc=nc,
        )
        return tuple(outs)

    devices = jax.devices()[:N_CORES]
    mesh = Mesh(np.asarray(devices), ("core",))
    in_specs = (PartitionSpec("core"),) * (n_params + n_outs)
    out_specs = (PartitionSpec("core"),) * n_outs
    sharded = jax.jit(
        shard_map(_body, mesh=mesh, in_specs=in_specs, out_specs=out_specs,
                  check_rep=False),
        donate_argnums=donate, keep_unused=True)

    def run(in_maps):
        concat_in = [
            np.concatenate([np.asarray(in_maps[c][name]) for c in range(N_CORES)],
                           axis=0)
            for name in in_names
        ]
        concat_zeros = [np.zeros((N_CORES * s[0], *s[1:]), dt)
                        for (s, dt) in zero_shapes]
        out_arrs = sharded(*concat_in, *concat_zeros)
        return [
            {name: np.asarray(out_arrs[i]).reshape(N_CORES, *out_avals[i].shape)[c]
             for i, name in enumerate(out_names)}
            for c in range(N_CORES)
        ]

    return run


def _get_runner():
    global _RUNNER
    if _RUNNER is None:
        _RUNNER = _make_runner()
    return _RUNNER


def _hilo16(v):
    """Split fp32 array v into bf16 hi/lo with hi+lo ~ v (double-bf16)."""
    hi = v.astype(BF16_NP)
    lo = (v - hi.astype(np.float32)).astype(BF16_NP)
    return hi, lo


def _hilo8(v):
    """Split fp32 vector v into fp8 hi/lo with hi+lo ~ v."""
    hi = v.astype(FP8_NP)
    lo = (v - hi.astype(np.float32)).astype(FP8_NP)
    return hi, lo


def make_in_maps(inputs, targets, centers):
    x = np.ascontiguousarray(np.asarray(inputs, dtype=np.float32))
    t = np.asarray(targets).astype(np.int64)
    c = np.ascontiguousarray(np.asarray(centers, dtype=np.float32))

    sqx = np.sum(x * x, axis=1, dtype=np.float32)          # [B]
    sqc = np.sum(c * c, axis=1, dtype=np.float32)          # [C]
    cnt = np.bincount(t, minlength=C).astype(np.float32)   # [C]
    absent = (cnt == 0).astype(np.float32)
    onehot = (t[:, None] == np.arange(C)[None, :]).astype(np.float32)  # [B, C]

    xtT = np.ascontiguousarray(x.T)                        # [D, B]
    ctT = np.ascontiguousarray(c.T)                        # [D, C]
    nI = ((-0.5 * DIAG_SQ) * np.eye(P)).astype(BF16_NP)

    # phase A operands (shared across cores except the x shard)
    cthi, ctlo = _hilo16(ctT)                              # [D, C]
    cthl_full = np.stack([cthi, ctlo], axis=1)             # [D, 2, C]
    cthl = np.ascontiguousarray(
        cthl_full.reshape(KT, P, 2, C).transpose(1, 0, 2, 3)
        .reshape(P, KT * 2 * C))
    bhi, blo = _hilo16(-0.5 * (sqc + BIG * absent))        # [C]
    ones_c = np.ones(C, BF16_NP)

    in_maps = []
    for core in range(N_CORES):
        off = core * SH
        ohr = np.roll(onehot, -off, axis=0)
        sqxr = np.roll(sqx, -off)
        xr_cols = np.roll(xtT, -off, axis=1)
        xt8 = np.ascontiguousarray(
            xr_cols.astype(FP8_NP).reshape(2, 2, P, B).transpose(0, 2, 1, 3))
        # norm rows scaled by 1/4 (fp8e4 saturates at 240; -0.5|x|^2 ~ -320),
        # with the constant 4.0 on the partner side so the product is exact
        hij, loj = _hilo8(-0.125 * sqxr)                   # [B]
        hii, loi = _hilo8(-0.125 * sqx[off:off + SH])      # [SH]
        four_b = np.full(B, 4.0, FP8_NP)
        four_s = np.full(SH, 4.0, FP8_NP)
        xe8 = np.stack([np.stack([hij, four_b]),
                        np.stack([loj, four_b])])          # [2, 2, B]
        xr8 = np.stack([np.stack([four_s, hii]),
                        np.stack([four_s, loi])])          # [2, 2, SH]

        xs = xtT[:, off:off + SH]                          # [D, SH]
        xshi, xslo = _hilo16(xs)
        xhi = np.ascontiguousarray(
            xshi.reshape(KT, P, SH).transpose(1, 0, 2).reshape(P, KT * SH))
        xlo = np.ascontiguousarray(
            xslo.reshape(KT, P, SH).transpose(1, 0, 2).reshape(P, KT * SH))
        ahi, alo = _hilo16(-0.5 * sqx[off:off + SH])       # [SH]
        ones_sh = np.ones(SH, BF16_NP)
        xab = np.zeros((4, SH + C), BF16_NP)
        xab[0, :SH] = ahi
        xab[1, :SH] = alo
        xab[2, :SH] = ones_sh
        xab[3, :SH] = ones_sh
        xab[0, SH:] = ones_c
        xab[1, SH:] = ones_c
        xab[2, SH:] = bhi
        xab[3, SH:] = blo

        ohs_t = np.ascontiguousarray(
            onehot[off:off + SH].reshape(NCH, P, C).transpose(1, 0, 2)
            .reshape(P, NCH * C))
        in_maps.append({
            "xt8": xt8,
            "xe8": np.ascontiguousarray(xe8),
            "xr8": np.ascontiguousarray(xr8),
            "ohp": np.ascontiguousarray(
                np.pad(ohr, ((0, 0), (0, CP - C)))
                .reshape(NJ, P, CP).transpose(1, 0, 2).reshape(P, NJ * CP)
            ).astype(FP8_NP),
            "nI": nI,
            "xhi": xhi,
            "xlo": xlo,
            "cthl": cthl,
            "xab": xab,
            "ohs": ohs_t,
            "ohsb": np.ascontiguousarray(BIG * ohs_t),
        })
    return in_maps


def finish(targets, per_core_out, per_core_out2):
    t = np.asarray(targets).astype(np.int64)
    cnt = np.bincount(t, minlength=C).astype(np.float64)

    outs = np.stack(per_core_out).astype(np.float64)       # [8, 128, 8]
    l1 = outs[:, :, 0:NCH].sum()
    pr = outs[:, :, NCH:2 * NCH].sum()

    st = np.stack(per_core_out2).astype(np.float64)        # [8, C, SH]
    s_full = st.transpose(0, 2, 1).reshape(B, C)           # [B, C] = S
    pos_sum = (s_full[np.arange(B), t] - 64.0) / SQ_SCALE  # diag contributes 64
    tot_sum = (s_full.sum(axis=1) - 64.0) / SQ_SCALE
    pos_cnt = cnt[t]
    pos_mean = pos_sum / pos_cnt
    neg_mean = (tot_sum - pos_sum) / (B - pos_cnt)
    l2 = (pos_mean + np.maximum(IAML_MARGIN - neg_mean, 0.0)).sum()

    loss = np.float32(l1 / B + 0.5 * (l2 / B))
    prec = np.float32(pr / B)
    return (np.asarray(loss, dtype=np.float32), np.asarray(prec, dtype=np.float32))


def kernel(inputs, targets, centers):
    in_maps = make_in_maps(inputs, targets, centers)
    try:
        results = _get_runner()(in_maps)
    except Exception:
        # one retry for transient device hiccups
        results = _get_runner()(in_maps)
    return finish(targets,
                  [results[i]["out"] for i in range(N_CORES)],
                  [results[i]["out2"] for i in range(N_CORES)])
